# revision 1
# baseline (speedup 1.0000x reference)
"""GLM-style dual-RoPE attention block on 8 trn2 NeuronCores.

Sharding: tensor-parallel over heads (16 heads -> 2 per core).
Per core: QKV projection for its heads (transposed layout), dual RoPE,
full S x S attention (streamed softmax over key tiles, no max subtraction
-- max |logit| ~60 so exp stays in fp32 range), unnormalized P@V,
late normalization, and a partial output projection.  Partials are summed
on host; qkv v-bias is folded into a host-side constant row, attn_out
bias added on host.

All matmuls run in bf16 (measured 227 ns per [128x128]@[128x512] vs
427 ns for f32r at ramped clock -- f32r is SBUF-bandwidth-bound at peak
frequency).  RoPE is computed as qk = (psum+b)*cos + swap32((psum+b)*sinP)
where sinP has the rotate-half sign folded in on host and swap32 is a
4x[32,512] SBUF->SBUF DMA partition swap -- no ACT-engine rotate copies.
The softmax denominator comes from a bf16 running accumulation of the
exp tiles on DVE plus two [1,512] ones-matmuls per (head, 1024-query
block), replacing the per-key-tile ones-matmuls (saves ~60k PE rows).
The output projection for each 1024-query block is emitted right after
that block's attention so the PE never drains until the very end.
"""

import ml_dtypes
import numpy as np

import concourse.bass as bass
from concourse import bacc
import concourse.mybir as mybir
import concourse.tile as tile
from concourse.bass_utils import run_bass_kernel_spmd
from concourse.masks import make_identity

F32 = mybir.dt.float32
BF16 = mybir.dt.bfloat16
F16 = mybir.dt.float16
AF = mybir.ActivationFunctionType
OP = mybir.AluOpType

S, D, H, HD = 2048, 2048, 16, 128
NCORES = 8
HPC = H // NCORES          # heads per core = 2
KT = D // 128              # 16 contraction tiles
ST = S // 128              # 16 sequence 128-tiles
QC = S // 512              # 4 sequence 512-chunks

_LAST_RESULTS = None
_BUILT = None


def _build():
    nc = bacc.Bacc("TRN2", target_bir_lowering=False, debug=False,
                   num_devices=NCORES)
    xT_d = nc.dram_tensor("xT", [D, S], F16, kind="ExternalInput").ap()
    wqk_d = nc.dram_tensor("wqk", [D, 4 * 128], F16, kind="ExternalInput").ap()
    bqk_d = nc.dram_tensor("bqk", [128, 4], F32, kind="ExternalInput").ap()
    wv_d = nc.dram_tensor("wv", [D, HPC * 128], F16, kind="ExternalInput").ap()
    cos_d = nc.dram_tensor("cos", [128, S], F16, kind="ExternalInput").ap()
    sin_d = nc.dram_tensor("sin", [128, S], F16, kind="ExternalInput").ap()
    wo_d = nc.dram_tensor("wo", [HPC * 128, D], BF16, kind="ExternalInput").ap()
    out_d = nc.dram_tensor("out", [S, D], BF16, kind="ExternalOutput").ap()

    with tile.TileContext(nc) as tc:
        with tc.tile_pool(name="res", bufs=1) as res:
            cos_sb = res.tile([128, S], F16, tag="cos")
            sin_sb = res.tile([128, S], F16, tag="sin")
            bqk_sb = res.tile([128, 4], F32, tag="bqk")
            wo_sb = [res.tile([128, D], BF16, tag=f"wo{h}", name=f"wo{h}")
                     for h in range(HPC)]
            qkT = [[res.tile([128, 512], F16, tag=f"qkT{m}_{nq}",
                             name=f"qkT{m}_{nq}") for nq in range(QC)]
                   for m in range(4)]
            vnat = [res.tile([128, 256], BF16, tag=f"vnat{st}",
                             name=f"vnat{st}") for st in range(ST)]
            wvn = [[res.tile([128, 512], BF16, tag=f"wvn{h}_{nq}",
                             name=f"wvn{h}_{nq}") for nq in range(QC)]
                   for h in range(HPC)]

            # ---- phase 1: qkv^T = W^T @ x^T (streamed over s-quarters) ----
            with (
                tc.tile_pool(name="xs", bufs=4) as xs,
                tc.tile_pool(name="tmp", bufs=3) as tmp,
                tc.tile_pool(name="vt", bufs=1) as vtp,
                tc.tile_pool(name="ps1", bufs=8, space="PSUM") as ps1,
            ):
                wqk_sb = [xs.tile([128, 512], F16, tag=f"wqk{k}",
                                  name=f"wqk{k}", bufs=1) for k in range(KT)]
                wv_sb = [xs.tile([128, 256], F16, tag=f"wvw{k}",
                                 name=f"wvw{k}", bufs=1) for k in range(KT)]
                vT = [[vtp.tile([128, 512], BF16, tag=f"vT{h}_{nq}",
                                name=f"vT{h}_{nq}") for nq in range(QC)]
                      for h in range(HPC)]
                ones_f = res.tile([128, 128], F32, tag="ones_f")
                nc.gpsimd.memset(ones_f[:], 1.0)
                ones_sb = res.tile([128, 128], BF16, tag="ones")
                nc.vector.tensor_copy(ones_sb[:], ones_f[:])
                ident_f = res.tile([128, 128], F32, tag="ident_f")
                make_identity(nc, ident_f[:])
                ident = res.tile([128, 128], BF16, tag="ident")
                nc.vector.tensor_copy(ident[:], ident_f[:])

                def rope_evict(m, nq, psums):
                    # single cheap op frees the PSUM bank in ~0.7us; the rest
                    # of the rope then runs from SBUF at 2x fp16 DVE rate
                    zb = tmp.tile([128, 512], F16, tag="zb", bufs=5,
                                  name=f"zb{m}_{nq}")
                    nc.vector.tensor_scalar_add(zb[:], psums[m][:],
                                                bqk_sb[:, m:m + 1])
                    return zb

                def rope_finish(m, nq, zb):
                    ns = slice(nq * 512, (nq + 1) * 512)
                    ts = tmp.tile([128, 512], F16, tag="ts", name=f"ts{m}_{nq}")
                    nc.vector.tensor_mul(ts[:], zb[:], sin_sb[:, ns])
                    tp = tmp.tile([128, 512], F16, tag="tp", name=f"tp{m}_{nq}")
                    for blk in range(2):
                        b0 = blk * 64
                        nc.gpsimd.dma_start(tp[b0:b0 + 32, :],
                                            ts[b0 + 32:b0 + 64, :])
                        nc.gpsimd.dma_start(tp[b0 + 32:b0 + 64, :],
                                            ts[b0:b0 + 32, :])
                    t1 = tmp.tile([128, 512], F16, tag="t1", name=f"t1{m}_{nq}")
                    nc.vector.tensor_mul(t1[:], zb[:], cos_sb[:, ns])
                    nc.vector.tensor_add(qkT[m][nq][:], t1[:], tp[:])

                for nq in range(QC):
                    ns = slice(nq * 512, (nq + 1) * 512)
                    psums = [ps1.tile([128, 512], F32, tag="ps",
                                      name=f"qkvps{nq}_{i}", bufs=8)
                             for i in range(6)]
                    for k in range(KT):
                        if nq == 0:
                            # weights on the scalar DMA queue so nq=0 is not
                            # paced by a single queue
                            nc.scalar.dma_start(wqk_sb[k][:],
                                                wqk_d[k * 128:(k + 1) * 128, :])
                            nc.scalar.dma_start(wv_sb[k][:],
                                                wv_d[k * 128:(k + 1) * 128, :])
                        xt = xs.tile([128, 512], F16, tag="xt", bufs=8)
                        nc.sync.dma_start(xt[:], xT_d[k * 128:(k + 1) * 128, ns])
                        if nq == 0 and k == 8:
                            # tables + wo on the gpsimd DMA queue, mid-way so
                            # they don't compete with the first xt tiles
                            nc.gpsimd.dma_start(cos_sb[:], cos_d[:, :])
                            nc.gpsimd.dma_start(sin_sb[:], sin_d[:, :])
                            nc.gpsimd.dma_start(bqk_sb[:], bqk_d[:, :])
                            for h in range(HPC):
                                nc.gpsimd.dma_start(
                                    wo_sb[h][:],
                                    wo_d[h * 128:(h + 1) * 128, :])
                        # v first in the last k-group: its psums close
                        # earliest so vT copies + transposes start sooner
                        order = ((4, 5, 2, 3, 0, 1) if k == KT - 1
                                 else (2, 3, 0, 1, 4, 5))
                        for m in order:
                            w = (wqk_sb[k][:, m * 128:(m + 1) * 128] if m < 4
                                 else wv_sb[k][:, (m - 4) * 128:
                                              (m - 3) * 128])
                            nc.tensor.matmul(
                                psums[m][:], w, xt[:],
                                start=(k == 0), stop=(k == KT - 1))
                    # evict all qk psums first (frees banks fastest), v next
                    zbs = {m: rope_evict(m, nq, psums) for m in (2, 3, 0, 1)}
                    for h in range(HPC):
                        nc.scalar.copy(vT[h][nq][:], psums[4 + h][:])
                    for h in range(HPC):
                        for j in range(4):
                            st = nq * 4 + j
                            tp2 = ps1.tile([128, 128], BF16, tag="ps", bufs=8,
                                           name=f"tp2_{h}_{st}")
                            nc.tensor.transpose(
                                tp2[:],
                                vT[h][nq][:, j * 128:(j + 1) * 128],
                                ident[:])
                            nc.scalar.copy(vnat[st][:, h * 128:(h + 1) * 128],
                                           tp2[:])
                    for m in (2, 3, 0, 1):
                        rope_finish(m, nq, zbs[m])

            # ---- phase 2 + 3: attention and out-projection, per 1024-q ----
            with (
                tc.tile_pool(name="ex", bufs=4) as exp_pool,
                tc.tile_pool(name="ac", bufs=2) as acp,
                tc.tile_pool(name="rp", bufs=2) as rp,
                tc.tile_pool(name="ob", bufs=2) as obp,
                tc.tile_pool(name="ps2", bufs=1, space="PSUM") as ps2,
                tc.tile_pool(name="ps3", bufs=2, space="PSUM") as ps3,
            ):
                # out-projection slots for completed 1024-q blocks; popped
                # one per key-tile inside later attention blocks so the PE
                # fills the exp-cadence stalls instead of idling
                ph3 = []
                obs = {}

                def emit_ph3():
                    if not ph3:
                        return
                    qt, oc = ph3.pop(0)
                    if oc == 0:
                        obs[qt] = obp.tile([128, D], BF16, tag="ob",
                                           name=f"ob{qt}")
                    ob = obs[qt]
                    op = ps3.tile([128, 512], F32, tag="op", bufs=2,
                                  name=f"op{qt}_{oc}")
                    for h2 in range(HPC):
                        nc.tensor.matmul(
                            op[:],
                            wvn[h2][qt // 4][:, (qt % 4) * 128:
                                             (qt % 4 + 1) * 128],
                            wo_sb[h2][:, oc * 512:(oc + 1) * 512],
                            start=(h2 == 0), stop=(h2 == HPC - 1))
                    if oc % 2 == 0:
                        nc.vector.tensor_copy(ob[:, oc * 512:(oc + 1) * 512],
                                              op[:])
                    else:
                        nc.scalar.copy(ob[:, oc * 512:(oc + 1) * 512], op[:])
                    if oc == 1:
                        nc.sync.dma_start(out_d[qt * 128:(qt + 1) * 128,
                                                0:1024], ob[:, 0:1024])
                    elif oc == 3:
                        nc.sync.dma_start(out_d[qt * 128:(qt + 1) * 128,
                                                1024:2048], ob[:, 1024:2048])
                        obs.pop(qt)

                for qc in range(2):
                    for h in range(HPC):
                        qT_h = qkT[h]
                        kT_h = qkT[2 + h]
                        wv_ps = ps2.tile([128, 1024], F32, tag="wv", bufs=1,
                                         name=f"wvps{h}_{qc}")
                        acc = acp.tile([128, 1024], BF16, tag="acc")
                        exs = {}
                        for st in range(ST + 2):   # 2-deep software pipeline
                            if st < ST:
                                lg = ps2.tile([128, 1024], F32, tag="lg",
                                              bufs=2, name=f"lg{h}_{qc}_{st}")
                                kts = kT_h[st // 4][:, (st % 4) * 128:
                                                    (st % 4 + 1) * 128]
                                for half in range(2):
                                    nc.tensor.matmul(
                                        lg[:, half * 512:(half + 1) * 512],
                                        kts,
                                        qT_h[2 * qc + half][:],
                                        start=True, stop=True)
                                ex = exp_pool.tile([128, 1024], BF16,
                                                   tag="ex")
                                nc.scalar.activation(ex[:], lg[:], AF.Exp)
                                if st == 0:
                                    nc.vector.tensor_copy(acc[:], ex[:])
                                else:
                                    nc.vector.tensor_add(acc[:], acc[:], ex[:])
                                exs[st] = ex
                            if st >= 2:
                                ex = exs.pop(st - 2)
                                sp = st - 2
                                for half in range(2):
                                    exh = ex[:, half * 512:(half + 1) * 512]
                                    nc.tensor.matmul(
                                        wv_ps[:, half * 512:(half + 1) * 512],
                                        vnat[sp][:, h * 128:(h + 1) * 128],
                                        exh,
                                        start=(sp == 0), stop=(sp == ST - 1))
                                if st >= 4:
                                    emit_ph3()
                            if st == ST + 1:
                                # denominator after PV(15) in queue order; the
                                # all-ones stationary reduces AND broadcasts
                                # across partitions in the matmul itself
                                sm = ps2.tile([128, 1024], F32, tag="lg",
                                              bufs=2, name=f"sm{h}_{qc}")
                                for half in range(2):
                                    nc.tensor.matmul(
                                        sm[:, half * 512:(half + 1) * 512],
                                        ones_sb[:],
                                        acc[:, half * 512:(half + 1) * 512],
                                        start=True, stop=True)
                                rc = rp.tile([128, 1024], F32, tag="rc",
                                             bufs=2, name=f"rc{h}_{qc}")
                                nc.vector.reciprocal_approx_fast(rc[:], sm[:])
                        for half in range(2):
                            nc.vector.tensor_mul(
                                wvn[h][2 * qc + half][:],
                                wv_ps[:, half * 512:(half + 1) * 512],
                                rc[:, half * 512:(half + 1) * 512])
                    ph3.extend((qt, oc)
                               for qt in range(8 * qc, 8 * qc + 8)
                               for oc in range(4))
                while ph3:
                    emit_ph3()

    nc.compile()
    return nc


def kernel(x, qkv_weight, qkv_bias, attn_out_weight, attn_out_bias,
           position_ids):
    global _BUILT, _LAST_RESULTS
    x = np.asarray(x, np.float32)
    qkv_weight = np.asarray(qkv_weight, np.float32)
    qkv_bias = np.asarray(qkv_bias, np.float32)
    attn_out_weight = np.asarray(attn_out_weight, np.float32)
    attn_out_bias = np.asarray(attn_out_bias, np.float32)
    position_ids = np.asarray(position_ids)

    half = HD // 2
    xT = np.ascontiguousarray(x[:, 0, :].T.astype(np.float16))
    inv_freq = 1.0 / (10000.0 ** (np.arange(0, half, 2, dtype=np.float32) / half))
    pos1 = position_ids[0, 0, :].astype(np.float32)
    pos2 = position_ids[0, 1, :].astype(np.float32)
    ang1 = np.concatenate([inv_freq[:, None] * pos1[None, :]] * 2, axis=0)
    ang2 = np.concatenate([inv_freq[:, None] * pos2[None, :]] * 2, axis=0)
    COS = np.concatenate([np.cos(ang1), np.cos(ang2)], axis=0)
    SIN = np.concatenate([np.sin(ang1), np.sin(ang2)], axis=0)
    # fold the rotate-half sign into sin: rows 32:64 and 96:128 negated
    sign = np.ones((128, 1), np.float32)
    sign[32:64] = -1.0
    sign[96:128] = -1.0
    SINP = np.ascontiguousarray((SIN * sign).astype(np.float16))
    COS = np.ascontiguousarray(COS.astype(np.float16))

    in_maps = []
    for c in range(NCORES):
        c0 = c * HPC * HD                     # first q column of this core
        wq = qkv_weight[:, c0:c0 + HPC * HD]
        wk = qkv_weight[:, D + c0:D + c0 + HPC * HD]
        wv = qkv_weight[:, 2 * D + c0:2 * D + c0 + HPC * HD]
        bq = qkv_bias[c0:c0 + HPC * HD]
        bk = qkv_bias[D + c0:D + c0 + HPC * HD]
        wo = attn_out_weight[c0:c0 + HPC * HD, :]
        wqk = np.ascontiguousarray(
            np.concatenate([wq, wk], axis=1).astype(np.float16))
        bqk = np.ascontiguousarray(
            np.stack([bq[:128], bq[128:], bk[:128], bk[128:]], axis=1))
        in_maps.append({
            "xT": xT,
            "wqk": wqk,
            "bqk": bqk,
            "wv": np.ascontiguousarray(wv.astype(np.float16)),
            "cos": COS,
            "sin": SINP,
            "wo": np.ascontiguousarray(wo.astype(ml_dtypes.bfloat16)),
        })

    if _BUILT is None:
        _BUILT = _build()
    res = run_bass_kernel_spmd(_BUILT, in_maps, core_ids=list(range(NCORES)))
    _LAST_RESULTS = res

    acc = np.zeros((S, D), dtype=np.float32)
    for r in res.results:
        acc += r["out"].astype(np.float32)
    bv = qkv_bias[2 * D:3 * D]
    acc += (bv @ attn_out_weight)[None, :] + attn_out_bias[None, :]
    return acc.reshape(S, 1, D).astype(np.float32)



# revision 17
# speedup vs baseline: 1.0565x; 1.0565x over previous
"""GLM-style dual-RoPE attention block on 8 trn2 NeuronCores.

Sharding: tensor-parallel over heads (16 heads -> 2 per core).
Per core: QKV projection for its heads (transposed layout), dual RoPE,
full S x S attention (streamed softmax over key tiles, no max subtraction
-- max |logit| ~60 so exp stays in bf16 range), unnormalized P@V,
late normalization, and a partial output projection.  Partials are summed
on host; qkv v-bias is folded into a host-side constant row, attn_out
bias added on host.

v2 restructure vs the 228us baseline:
- Phase 1 runs m-outer / k-inner: each of the 6 qkv output groups
  accumulates over all 16 contraction tiles before the next group
  starts, so PSUM evictions trail one group behind the matmul stream
  and the 2-3us round-boundary stalls disappear.  Weights are packed
  per-m on host so a group's stationary tiles arrive in 2 big DMAs.
- x tiles stream on two DMA queues (sync + gpsimd) to double round-0
  fill bandwidth; cos/sin/wo arrive later, off the critical window.
- Attention uses 512-query blocks (4 qc x 2 heads).  Logits go to
  [128,1024] f32 PSUM pairs (two key tiles), one Exp per pair, the
  bf16 exp accumulation is split across vector (even tile) and gpsimd
  (odd tile) with the two partial sums folded by an accumulating
  ones-matmul pair.  PV lags logits by 2 tiles.
- Output projection slots (qt, oc) are popped one per st-pair as soon
  as a query block's two heads are normalized, so only the last query
  block's 16 slots drain at the end (vs 36 before); drain evictions
  alternate vector/scalar.
"""

import ml_dtypes
import numpy as np

import concourse.bass as bass
from concourse import bacc
import concourse.mybir as mybir
import concourse.tile as tile
from concourse.bass_utils import run_bass_kernel_spmd
from concourse.masks import make_identity

F32 = mybir.dt.float32
BF16 = mybir.dt.bfloat16
F16 = mybir.dt.float16
AF = mybir.ActivationFunctionType
OP = mybir.AluOpType

S, D, H, HD = 2048, 2048, 16, 128
NCORES = 8
HPC = H // NCORES          # heads per core = 2
KT = D // 128              # 16 contraction tiles
ST = S // 128              # 16 sequence 128-tiles
QC = S // 512              # 4 sequence 512-chunks

_LAST_RESULTS = None
_BUILT = None


def _build():
    nc = bacc.Bacc("TRN2", target_bir_lowering=False, debug=False,
                   num_devices=NCORES)
    xT_d = nc.dram_tensor("xT", [D, S], F16, kind="ExternalInput").ap()
    # wqk packed per-m: [128, 4*2048], m-th slice's col k*128.. holds
    # W[k*128:(k+1)*128, m-slice] (stationary tiles contiguous per m)
    wqk_d = nc.dram_tensor("wqk", [128, 4 * D], F16, kind="ExternalInput").ap()
    bqk_d = nc.dram_tensor("bqk", [128, 4], F32, kind="ExternalInput").ap()
    wv_d = nc.dram_tensor("wv", [128, HPC * D], F16, kind="ExternalInput").ap()
    cos_d = nc.dram_tensor("cos", [128, S], F16, kind="ExternalInput").ap()
    sin_d = nc.dram_tensor("sin", [128, S], F16, kind="ExternalInput").ap()
    wo_d = nc.dram_tensor("wo", [HPC * 128, D], BF16, kind="ExternalInput").ap()
    out_d = nc.dram_tensor("out", [S, D], BF16, kind="ExternalOutput").ap()

    with tile.TileContext(nc) as tc:
        with tc.tile_pool(name="res", bufs=1) as res:
            cos_sb = res.tile([128, S], F16, tag="cos")
            sin_sb = res.tile([128, S], F16, tag="sin")
            bqk_sb = res.tile([128, 4], F32, tag="bqk")
            wo_sb = [res.tile([128, D], BF16, tag=f"wo{h}", name=f"wo{h}")
                     for h in range(HPC)]
            qkT = [[res.tile([128, 512], F16, tag=f"qkT{m}_{nq}",
                             name=f"qkT{m}_{nq}") for nq in range(QC)]
                   for m in range(4)]
            vnat = [res.tile([128, 256], BF16, tag=f"vnat{st}",
                             name=f"vnat{st}") for st in range(ST)]
            wvn = [[res.tile([128, 512], BF16, tag=f"wvn{h}_{nq}",
                             name=f"wvn{h}_{nq}") for nq in range(QC)]
                   for h in range(HPC)]

            # ---- phase 1: qkv^T = W^T @ x^T, m-outer ----
            with (
                tc.tile_pool(name="xs", bufs=1) as xs,
                tc.tile_pool(name="tmp", bufs=3) as tmp,
                tc.tile_pool(name="vt", bufs=1) as vtp,
                tc.tile_pool(name="ps1", bufs=5, space="PSUM") as ps1,
                tc.tile_pool(name="pst", bufs=2, space="PSUM") as pst,
            ):
                # m=0 in quarters so the very first matmul only waits on
                # a 128KB transfer; m=1..3 in halves
                wqk_sb = [[xs.tile([128, 512 if m == 0 else 1024], F16,
                                   tag=f"wqk{m}_{ha}",
                                   name=f"wqk{m}_{ha}", bufs=1)
                           for ha in range(4 if m == 0 else 2)]
                          for m in range(4)]
                wv_sb = [[xs.tile([128, 1024], F16, tag=f"wvw{h}_{ha}",
                                  name=f"wvw{h}_{ha}", bufs=1)
                          for ha in range(2)] for h in range(HPC)]
                vT = [[vtp.tile([128, 512], BF16, tag=f"vT{h}_{nq}",
                                name=f"vT{h}_{nq}") for nq in range(QC)]
                      for h in range(HPC)]
                ones_f = res.tile([128, 128], F32, tag="ones_f")
                nc.gpsimd.memset(ones_f[:], 1.0)
                ones_sb = res.tile([128, 128], BF16, tag="ones")
                nc.vector.tensor_copy(ones_sb[:], ones_f[:])
                ident_f = res.tile([128, 128], F32, tag="ident_f")
                make_identity(nc, ident_f[:])
                ident = res.tile([128, 128], BF16, tag="ident")
                nc.vector.tensor_copy(ident[:], ident_f[:])

                def rope_evict(m, nq, ps):
                    zb = tmp.tile([128, 512], F16, tag="zb", bufs=6,
                                  name=f"zb{m}_{nq}")
                    nc.vector.tensor_scalar_add(zb[:], ps[:],
                                                bqk_sb[:, m:m + 1])
                    return zb

                def rope_finish(m, nq, zb):
                    ns = slice(nq * 512, (nq + 1) * 512)
                    ts = tmp.tile([128, 512], F16, tag="ts", bufs=6,
                                  name=f"ts{m}_{nq}")
                    nc.vector.tensor_mul(ts[:], zb[:], sin_sb[:, ns])
                    tp = tmp.tile([128, 512], F16, tag="tp", bufs=6,
                                  name=f"tp{m}_{nq}")
                    for blk in range(2):
                        b0 = blk * 64
                        nc.gpsimd.dma_start(tp[b0:b0 + 32, :],
                                            ts[b0 + 32:b0 + 64, :])
                        nc.gpsimd.dma_start(tp[b0 + 32:b0 + 64, :],
                                            ts[b0:b0 + 32, :])
                    t1 = tmp.tile([128, 512], F16, tag="t1",
                                  name=f"t1{m}_{nq}")
                    nc.vector.tensor_mul(t1[:], zb[:], cos_sb[:, ns])
                    nc.vector.tensor_add(qkT[m][nq][:], t1[:], tp[:])

                xts = {}

                def issue_xt(nq, k):
                    t = xs.tile([128, 512], F16, tag="xt", bufs=32,
                                name=f"xt{nq}_{k}")
                    q = nc.sync if (k % 2 == 0) else nc.gpsimd
                    q.dma_start(t[:], xT_d[k * 128:(k + 1) * 128,
                                           nq * 512:(nq + 1) * 512])
                    xts[(nq, k)] = t

                def _emit_T(h, nq):
                    for j in range(4):
                        st = nq * 4 + j
                        tp2 = pst.tile([128, 128], BF16, tag="tp2", bufs=2,
                                       name=f"tp2_{h}_{st}")
                        nc.tensor.transpose(
                            tp2[:], vT[h][nq][:, j * 128:(j + 1) * 128],
                            ident[:])
                        nc.scalar.copy(vnat[st][:, h * 128:(h + 1) * 128],
                                       tp2[:])

                pending_T = []
                deferred_rope = []

                for nq in range(QC):
                    for k in range(KT):
                        issue_xt(nq, k)
                    if nq == 0:
                        # bqk first (needed at the first evict); weights
                        # on the scalar queue, rope tables on gpsimd
                        # (after its xt issues) -- only sync/scalar/
                        # gpsimd can host DMA rings
                        nc.scalar.dma_start(bqk_sb[:], bqk_d[:, :])
                        for m in range(4):
                            npiece = 4 if m == 0 else 2
                            w = 2048 // npiece
                            for ha in range(npiece):
                                nc.scalar.dma_start(
                                    wqk_sb[m][ha][:],
                                    wqk_d[:, m * D + ha * w:
                                          m * D + (ha + 1) * w])
                        for h in range(HPC):
                            for ha in range(2):
                                nc.scalar.dma_start(
                                    wv_sb[h][ha][:],
                                    wv_d[:, h * D + ha * 1024:
                                         h * D + (ha + 1) * 1024])
                        nc.gpsimd.dma_start(cos_sb[:], cos_d[:, :])
                        nc.gpsimd.dma_start(sin_sb[:], sin_d[:, :])
                    if nq == 1:
                        for h in range(HPC):
                            nc.gpsimd.dma_start(
                                wo_sb[h][:], wo_d[h * 128:(h + 1) * 128, :])
                    for m in range(6):
                        ps = ps1.tile([128, 512], F32, tag="qkv", bufs=5,
                                      name=f"qkv{nq}_{m}")
                        for k in range(KT):
                            if m == 0:
                                w = wqk_sb[0][k // 4][:, (k % 4) * 128:
                                                      (k % 4 + 1) * 128]
                            elif m < 4:
                                w = wqk_sb[m][k // 8][:, (k % 8) * 128:
                                                      (k % 8 + 1) * 128]
                            else:
                                w = wv_sb[m - 4][k // 8][:, (k % 8) * 128:
                                                         (k % 8 + 1) * 128]
                            nc.tensor.matmul(ps[:], w, xts[(nq, k)][:],
                                             start=(k == 0), stop=(k == KT - 1))
                        # flush v-transposes pending from the previous
                        # group so their scalar copies had time to land
                        for t_args in pending_T:
                            _emit_T(*t_args)
                        pending_T = []
                        if m < 4:
                            zb = rope_evict(m, nq, ps)
                            if nq == 0:
                                deferred_rope.append((m, nq, zb))
                            else:
                                rope_finish(m, nq, zb)
                        else:
                            h = m - 4
                            nc.scalar.copy(vT[h][nq][:], ps[:])
                            pending_T.append((h, nq))
                    if nq == 0:
                        # round 0: rope tables may still be in flight
                        # during the early groups; run theropes after
                        # the round's matmuls so vector work overlaps
                        # round 1's stream
                        for args in deferred_rope:
                            rope_finish(*args)
                        deferred_rope = []

                for t_args in pending_T:
                    _emit_T(*t_args)
                pending_T = []

            # ---- phase 2 + 3: attention and out-projection ----
            ph3 = []
            obs = {}
            nslot = [0]

            def emit_slot(oppool, opbufs, obpool):
                if not ph3:
                    return
                qt, oc = ph3.pop(0)
                if oc == 0:
                    obs[qt] = obpool.tile([128, D], BF16, tag="ob",
                                          name=f"ob{qt}")
                ob = obs[qt]
                op = oppool.tile([128, 512], F32, tag="op", bufs=opbufs,
                                 name=f"op{qt}_{oc}")
                for h2 in range(HPC):
                    nc.tensor.matmul(
                        op[:],
                        wvn[h2][qt // 4][:, (qt % 4) * 128:
                                         (qt % 4 + 1) * 128],
                        wo_sb[h2][:, oc * 512:(oc + 1) * 512],
                        start=(h2 == 0), stop=(h2 == HPC - 1))
                n = nslot[0]
                nslot[0] += 1
                use_scalar = (n % 2 == 1)
                dst = ob[:, oc * 512:(oc + 1) * 512]
                if use_scalar:
                    nc.scalar.copy(dst, op[:])
                else:
                    nc.vector.tensor_copy(dst, op[:])
                if oc == 1:
                    nc.sync.dma_start(out_d[qt * 128:(qt + 1) * 128,
                                            0:1024], ob[:, 0:1024])
                elif oc == 3:
                    nc.sync.dma_start(out_d[qt * 128:(qt + 1) * 128,
                                            1024:2048], ob[:, 1024:2048])
                    obs.pop(qt)

            with (
                tc.tile_pool(name="ex", bufs=3) as exp_pool,
                tc.tile_pool(name="ac", bufs=3) as acp,
                tc.tile_pool(name="rp", bufs=2) as rp,
                tc.tile_pool(name="ob", bufs=2) as obp,
                tc.tile_pool(name="ps2", bufs=2, space="PSUM") as ps2,
                tc.tile_pool(name="psm", bufs=1, space="PSUM") as psm,
                tc.tile_pool(name="psw", bufs=1, space="PSUM") as psw,
                tc.tile_pool(name="ps3", bufs=2, space="PSUM") as ps3,
            ):
                for qc in range(QC):
                    for h in range(HPC):
                        qT = qkT[h][qc]
                        kT_h = qkT[2 + h]
                        wv_ps = psw.tile([128, 512], F32, tag="wv", bufs=1,
                                         name=f"wvps{h}_{qc}")
                        exs = {}
                        acc = None
                        lg_cur = None
                        for st in range(ST + 2):
                            if st < ST:
                                pair, side = st // 2, st % 2
                                if side == 0:
                                    lg_cur = ps2.tile(
                                        [128, 1024], F32, tag="lg", bufs=2,
                                        name=f"lg{h}_{qc}_{pair}")
                                kts = kT_h[st // 4][:, (st % 4) * 128:
                                                    (st % 4 + 1) * 128]
                                nc.tensor.matmul(
                                    lg_cur[:, side * 512:(side + 1) * 512],
                                    kts, qT[:], start=True, stop=True)
                                if side == 1:
                                    ex = exp_pool.tile([128, 1024], BF16,
                                                       tag="ex", bufs=3,
                                                       name=f"ex{h}_{qc}_{pair}")
                                    nc.scalar.activation(ex[:], lg_cur[:],
                                                         AF.Exp)
                                    exs[pair] = ex
                                    # non-inplace running sum of the exp
                                    # pair tiles (chain of 7 adds, no copy)
                                    if pair == 1:
                                        nacc = acp.tile(
                                            [128, 1024], BF16, tag="acc",
                                            bufs=3, name=f"acc{h}_{qc}_1")
                                        nc.vector.tensor_add(
                                            nacc[:], exs[0][:], ex[:])
                                        acc = nacc
                                    elif pair > 1:
                                        nacc = acp.tile(
                                            [128, 1024], BF16, tag="acc",
                                            bufs=3,
                                            name=f"acc{h}_{qc}_{pair}")
                                        nc.vector.tensor_add(
                                            nacc[:], acc[:], ex[:])
                                        acc = nacc
                            if st >= 2:
                                sp = st - 2
                                ex = exs[sp // 2]
                                exh = ex[:, (sp % 2) * 512:
                                         (sp % 2 + 1) * 512]
                                nc.tensor.matmul(
                                    wv_ps[:],
                                    vnat[sp][:, h * 128:(h + 1) * 128],
                                    exh, start=(sp == 0), stop=(sp == ST - 1))
                                if sp % 2 == 1:
                                    exs.pop(sp // 2, None)
                                if st % 2 == 1:
                                    emit_slot(ps3, 2, obp)
                            if st == ST + 1:
                                sm = psm.tile([128, 512], F32, tag="sm",
                                              bufs=1, name=f"sm{h}_{qc}")
                                nc.tensor.matmul(sm[:], ones_sb[:],
                                                 acc[:, 0:512],
                                                 start=True, stop=False)
                                nc.tensor.matmul(sm[:], ones_sb[:],
                                                 acc[:, 512:1024],
                                                 start=False, stop=True)
                                rc = rp.tile([128, 512], F32, tag="rc",
                                             bufs=2, name=f"rc{h}_{qc}")
                                nc.vector.reciprocal_approx_fast(rc[:], sm[:])
                        nc.vector.tensor_mul(wvn[h][qc][:], wv_ps[:], rc[:])
                    ph3.extend((qc * 4 + j, oc)
                               for j in range(4) for oc in range(4))

            # final drain: attention PSUM pools are closed, so the last
            # query block's out-projections get a deep bank rotation
            assert not obs, "ob tile straddles the drain boundary"
            with (
                tc.tile_pool(name="obd", bufs=2) as obd,
                tc.tile_pool(name="psd", bufs=6, space="PSUM") as psd,
            ):
                while ph3:
                    emit_slot(psd, 6, obd)

    nc.compile()
    return nc


def kernel(x, qkv_weight, qkv_bias, attn_out_weight, attn_out_bias,
           position_ids):
    global _BUILT, _LAST_RESULTS
    x = np.asarray(x, np.float32)
    qkv_weight = np.asarray(qkv_weight, np.float32)
    qkv_bias = np.asarray(qkv_bias, np.float32)
    attn_out_weight = np.asarray(attn_out_weight, np.float32)
    attn_out_bias = np.asarray(attn_out_bias, np.float32)
    position_ids = np.asarray(position_ids)

    half = HD // 2
    xT = np.ascontiguousarray(x[:, 0, :].T.astype(np.float16))
    inv_freq = 1.0 / (10000.0 ** (np.arange(0, half, 2, dtype=np.float32) / half))
    pos1 = position_ids[0, 0, :].astype(np.float32)
    pos2 = position_ids[0, 1, :].astype(np.float32)
    ang1 = np.concatenate([inv_freq[:, None] * pos1[None, :]] * 2, axis=0)
    ang2 = np.concatenate([inv_freq[:, None] * pos2[None, :]] * 2, axis=0)
    COS = np.concatenate([np.cos(ang1), np.cos(ang2)], axis=0)
    SIN = np.concatenate([np.sin(ang1), np.sin(ang2)], axis=0)
    # fold the rotate-half sign into sin: rows 32:64 and 96:128 negated
    sign = np.ones((128, 1), np.float32)
    sign[32:64] = -1.0
    sign[96:128] = -1.0
    SINP = np.ascontiguousarray((SIN * sign).astype(np.float16))
    COS = np.ascontiguousarray(COS.astype(np.float16))

    in_maps = []
    for c in range(NCORES):
        c0 = c * HPC * HD                     # first q column of this core
        wq = qkv_weight[:, c0:c0 + HPC * HD]
        wk = qkv_weight[:, D + c0:D + c0 + HPC * HD]
        wv = qkv_weight[:, 2 * D + c0:2 * D + c0 + HPC * HD]
        bq = qkv_bias[c0:c0 + HPC * HD]
        bk = qkv_bias[D + c0:D + c0 + HPC * HD]
        wo = attn_out_weight[c0:c0 + HPC * HD, :]
        wqk = np.concatenate([wq, wk], axis=1).astype(np.float16)  # [D, 512]
        # pack stationary tiles per-m: [128, 4*2048]
        wqkP = np.concatenate(
            [np.concatenate([wqk[k * 128:(k + 1) * 128,
                                 m * 128:(m + 1) * 128]
                             for k in range(KT)], axis=1)
             for m in range(4)], axis=1)
        wvf = wv.astype(np.float16)
        wvP = np.concatenate(
            [np.concatenate([wvf[k * 128:(k + 1) * 128,
                                 h * 128:(h + 1) * 128]
                             for k in range(KT)], axis=1)
             for h in range(HPC)], axis=1)
        bqk = np.ascontiguousarray(
            np.stack([bq[:128], bq[128:], bk[:128], bk[128:]], axis=1))
        in_maps.append({
            "xT": xT,
            "wqk": np.ascontiguousarray(wqkP),
            "bqk": bqk,
            "wv": np.ascontiguousarray(wvP),
            "cos": COS,
            "sin": SINP,
            "wo": np.ascontiguousarray(wo.astype(ml_dtypes.bfloat16)),
        })

    if _BUILT is None:
        _BUILT = _build()
    res = run_bass_kernel_spmd(_BUILT, in_maps, core_ids=list(range(NCORES)))
    _LAST_RESULTS = res

    acc = np.zeros((S, D), dtype=np.float32)
    for r in res.results:
        acc += r["out"].astype(np.float32)
    bv = qkv_bias[2 * D:3 * D]
    acc += (bv @ attn_out_weight)[None, :] + attn_out_bias[None, :]
    return acc.reshape(S, 1, D).astype(np.float32)


# revision 23
# speedup vs baseline: 1.0611x; 1.0043x over previous
"""GLM-style dual-RoPE attention block on 8 trn2 NeuronCores.

Sharding: tensor-parallel over heads (16 heads -> 2 per core).
Per core: QKV projection for its heads (transposed layout), dual RoPE,
full S x S attention (streamed softmax over key tiles, no max subtraction
-- max |logit| ~60 so exp stays in bf16 range), unnormalized P@V,
late normalization, and a partial output projection.  Partials are summed
on host; qkv v-bias is folded into a host-side constant row, attn_out
bias added on host.

v2 restructure vs the 228us baseline:
- Phase 1 runs m-outer / k-inner: each of the 6 qkv output groups
  accumulates over all 16 contraction tiles before the next group
  starts, so PSUM evictions trail one group behind the matmul stream
  and the 2-3us round-boundary stalls disappear.  Weights are packed
  per-m on host so a group's stationary tiles arrive in 2 big DMAs.
- x tiles stream on two DMA queues (sync + gpsimd) to double round-0
  fill bandwidth; cos/sin/wo arrive later, off the critical window.
- Attention uses 512-query blocks (4 qc x 2 heads).  Logits go to
  [128,1024] f32 PSUM pairs (two key tiles), one Exp per pair, the
  bf16 exp accumulation is split across vector (even tile) and gpsimd
  (odd tile) with the two partial sums folded by an accumulating
  ones-matmul pair.  PV lags logits by 2 tiles.
- Output projection slots (qt, oc) are popped one per st-pair as soon
  as a query block's two heads are normalized, so only the last query
  block's 16 slots drain at the end (vs 36 before); drain evictions
  alternate vector/scalar.
"""

import ml_dtypes
import numpy as np

import concourse.bass as bass
from concourse import bacc
import concourse.mybir as mybir
import concourse.tile as tile
from concourse.bass_utils import run_bass_kernel_spmd
from concourse.masks import make_identity

F32 = mybir.dt.float32
BF16 = mybir.dt.bfloat16
F16 = mybir.dt.float16
AF = mybir.ActivationFunctionType
OP = mybir.AluOpType

S, D, H, HD = 2048, 2048, 16, 128
NCORES = 8
HPC = H // NCORES          # heads per core = 2
KT = D // 128              # 16 contraction tiles
ST = S // 128              # 16 sequence 128-tiles
QC = S // 512              # 4 sequence 512-chunks

_LAST_RESULTS = None
_BUILT = None


def _build():
    nc = bacc.Bacc("TRN2", target_bir_lowering=False, debug=False,
                   num_devices=NCORES)
    xT_d = nc.dram_tensor("xT", [D, S], F16, kind="ExternalInput").ap()
    # wqk packed per-m: [128, 4*2048], m-th slice's col k*128.. holds
    # W[k*128:(k+1)*128, m-slice] (stationary tiles contiguous per m)
    wqk_d = nc.dram_tensor("wqk", [128, 4 * D], F16, kind="ExternalInput").ap()
    bqk_d = nc.dram_tensor("bqk", [128, 4], F32, kind="ExternalInput").ap()
    wv_d = nc.dram_tensor("wv", [128, HPC * D], F16, kind="ExternalInput").ap()
    cos_d = nc.dram_tensor("cos", [128, S], F16, kind="ExternalInput").ap()
    sin_d = nc.dram_tensor("sin", [128, S], F16, kind="ExternalInput").ap()
    wo_d = nc.dram_tensor("wo", [HPC * 128, D], BF16, kind="ExternalInput").ap()
    out_d = nc.dram_tensor("out", [S, D], BF16, kind="ExternalOutput").ap()

    with tile.TileContext(nc) as tc:
        with tc.tile_pool(name="res", bufs=1) as res:
            cos_sb = res.tile([128, S], F16, tag="cos")
            sin_sb = res.tile([128, S], F16, tag="sin")
            bqk_sb = res.tile([128, 4], F32, tag="bqk")
            wo_sb = [res.tile([128, D], BF16, tag=f"wo{h}", name=f"wo{h}")
                     for h in range(HPC)]
            qkT = [[res.tile([128, 512], F16, tag=f"qkT{m}_{nq}",
                             name=f"qkT{m}_{nq}") for nq in range(QC)]
                   for m in range(4)]
            vnat = [res.tile([128, 256], BF16, tag=f"vnat{st}",
                             name=f"vnat{st}") for st in range(ST)]
            wvn = [[res.tile([128, 512], BF16, tag=f"wvn{h}_{nq}",
                             name=f"wvn{h}_{nq}") for nq in range(QC)]
                   for h in range(HPC)]

            # ---- phase 1: qkv^T = W^T @ x^T, m-outer ----
            with (
                tc.tile_pool(name="xs", bufs=1) as xs,
                tc.tile_pool(name="tmp", bufs=3) as tmp,
                tc.tile_pool(name="vt", bufs=1) as vtp,
                tc.tile_pool(name="ps1", bufs=5, space="PSUM") as ps1,
                tc.tile_pool(name="pst", bufs=2, space="PSUM") as pst,
            ):
                # m=0 in quarters so the very first matmul only waits on
                # a 128KB transfer; m=1..3 in halves
                wqk_sb = [[xs.tile([128, 512 if m == 0 else 1024], F16,
                                   tag=f"wqk{m}_{ha}",
                                   name=f"wqk{m}_{ha}", bufs=1)
                           for ha in range(4 if m == 0 else 2)]
                          for m in range(4)]
                wv_sb = [[xs.tile([128, 1024], F16, tag=f"wvw{h}_{ha}",
                                  name=f"wvw{h}_{ha}", bufs=1)
                          for ha in range(2)] for h in range(HPC)]
                vT = [[vtp.tile([128, 512], BF16, tag=f"vT{h}_{nq}",
                                name=f"vT{h}_{nq}") for nq in range(QC)]
                      for h in range(HPC)]
                ones_f = res.tile([128, 128], F32, tag="ones_f")
                nc.gpsimd.memset(ones_f[:], 1.0)
                ones_sb = res.tile([128, 128], BF16, tag="ones")
                nc.vector.tensor_copy(ones_sb[:], ones_f[:])
                ident_f = res.tile([128, 128], F32, tag="ident_f")
                make_identity(nc, ident_f[:])
                ident = res.tile([128, 128], BF16, tag="ident")
                nc.vector.tensor_copy(ident[:], ident_f[:])

                def rope_evict(m, nq, ps):
                    zb = tmp.tile([128, 512], F16, tag="zb", bufs=6,
                                  name=f"zb{m}_{nq}")
                    nc.vector.tensor_scalar_add(zb[:], ps[:],
                                                bqk_sb[:, m:m + 1])
                    return zb

                def rope_finish(m, nq, zb):
                    ns = slice(nq * 512, (nq + 1) * 512)
                    ts = tmp.tile([128, 512], F16, tag="ts", bufs=6,
                                  name=f"ts{m}_{nq}")
                    nc.vector.tensor_mul(ts[:], zb[:], sin_sb[:, ns])
                    tp = tmp.tile([128, 512], F16, tag="tp", bufs=6,
                                  name=f"tp{m}_{nq}")
                    for blk in range(2):
                        b0 = blk * 64
                        nc.gpsimd.dma_start(tp[b0:b0 + 32, :],
                                            ts[b0 + 32:b0 + 64, :])
                        nc.gpsimd.dma_start(tp[b0 + 32:b0 + 64, :],
                                            ts[b0:b0 + 32, :])
                    t1 = tmp.tile([128, 512], F16, tag="t1",
                                  name=f"t1{m}_{nq}")
                    nc.vector.tensor_mul(t1[:], zb[:], cos_sb[:, ns])
                    nc.vector.tensor_add(qkT[m][nq][:], t1[:], tp[:])

                xts = {}

                def issue_xt(nq, k):
                    t = xs.tile([128, 512], F16, tag="xt", bufs=32,
                                name=f"xt{nq}_{k}")
                    q = nc.sync if (k % 2 == 0) else nc.gpsimd
                    q.dma_start(t[:], xT_d[k * 128:(k + 1) * 128,
                                           nq * 512:(nq + 1) * 512])
                    xts[(nq, k)] = t

                def _emit_T(h, nq):
                    for j in range(4):
                        st = nq * 4 + j
                        tp2 = pst.tile([128, 128], BF16, tag="tp2", bufs=2,
                                       name=f"tp2_{h}_{st}")
                        nc.tensor.transpose(
                            tp2[:], vT[h][nq][:, j * 128:(j + 1) * 128],
                            ident[:])
                        nc.scalar.copy(vnat[st][:, h * 128:(h + 1) * 128],
                                       tp2[:])

                pending_T = []
                deferred_rope = []

                for nq in range(QC):
                    for k in range(KT):
                        issue_xt(nq, k)
                    if nq == 0:
                        # bqk first (needed at the first evict); weights
                        # on the scalar queue, rope tables on gpsimd
                        # (after its xt issues) -- only sync/scalar/
                        # gpsimd can host DMA rings
                        for m in range(4):
                            npiece = 4 if m == 0 else 2
                            w = 2048 // npiece
                            for ha in range(npiece):
                                nc.scalar.dma_start(
                                    wqk_sb[m][ha][:],
                                    wqk_d[:, m * D + ha * w:
                                          m * D + (ha + 1) * w])
                                if m == 0 and ha == 0:
                                    # tiny; needed first at m0's evict,
                                    # must not gate the first matmul
                                    nc.scalar.dma_start(bqk_sb[:],
                                                        bqk_d[:, :])
                        for h in range(HPC):
                            for ha in range(2):
                                nc.scalar.dma_start(
                                    wv_sb[h][ha][:],
                                    wv_d[:, h * D + ha * 1024:
                                         h * D + (ha + 1) * 1024])
                        nc.gpsimd.dma_start(cos_sb[:], cos_d[:, :])
                        nc.gpsimd.dma_start(sin_sb[:], sin_d[:, :])
                    if nq == 1:
                        for h in range(HPC):
                            nc.gpsimd.dma_start(
                                wo_sb[h][:], wo_d[h * 128:(h + 1) * 128, :])
                    for m in range(6):
                        ps = ps1.tile([128, 512], F32, tag="qkv", bufs=5,
                                      name=f"qkv{nq}_{m}")
                        for k in range(KT):
                            if m == 0:
                                w = wqk_sb[0][k // 4][:, (k % 4) * 128:
                                                      (k % 4 + 1) * 128]
                            elif m < 4:
                                w = wqk_sb[m][k // 8][:, (k % 8) * 128:
                                                      (k % 8 + 1) * 128]
                            else:
                                w = wv_sb[m - 4][k // 8][:, (k % 8) * 128:
                                                         (k % 8 + 1) * 128]
                            nc.tensor.matmul(ps[:], w, xts[(nq, k)][:],
                                             start=(k == 0), stop=(k == KT - 1))
                        # flush v-transposes pending from the previous
                        # group so their scalar copies had time to land
                        for t_args in pending_T:
                            _emit_T(*t_args)
                        pending_T = []
                        if m < 4:
                            zb = rope_evict(m, nq, ps)
                            if nq == 0:
                                deferred_rope.append((m, nq, zb))
                            else:
                                rope_finish(m, nq, zb)
                        else:
                            h = m - 4
                            nc.scalar.copy(vT[h][nq][:], ps[:])
                            pending_T.append((h, nq))
                    if nq == 0:
                        # round 0: rope tables may still be in flight
                        # during the early groups; run theropes after
                        # the round's matmuls so vector work overlaps
                        # round 1's stream
                        for args in deferred_rope:
                            rope_finish(*args)
                        deferred_rope = []

                for t_args in pending_T:
                    _emit_T(*t_args)
                pending_T = []

            # ---- phase 2 + 3: attention and out-projection ----
            ph3 = []
            obs = {}
            nslot = [0]

            def emit_slot(oppool, opbufs, obpool):
                if not ph3:
                    return
                qt, oc = ph3.pop(0)
                if oc == 0:
                    obs[qt] = obpool.tile([128, D], BF16, tag="ob",
                                          name=f"ob{qt}")
                ob = obs[qt]
                op = oppool.tile([128, 512], F32, tag="op", bufs=opbufs,
                                 name=f"op{qt}_{oc}")
                for h2 in range(HPC):
                    nc.tensor.matmul(
                        op[:],
                        wvn[h2][qt // 4][:, (qt % 4) * 128:
                                         (qt % 4 + 1) * 128],
                        wo_sb[h2][:, oc * 512:(oc + 1) * 512],
                        start=(h2 == 0), stop=(h2 == HPC - 1))
                n = nslot[0]
                nslot[0] += 1
                use_scalar = (n % 2 == 1)
                dst = ob[:, oc * 512:(oc + 1) * 512]
                if use_scalar:
                    nc.scalar.copy(dst, op[:])
                else:
                    nc.vector.tensor_copy(dst, op[:])
                if oc == 1:
                    nc.sync.dma_start(out_d[qt * 128:(qt + 1) * 128,
                                            0:1024], ob[:, 0:1024])
                elif oc == 3:
                    nc.sync.dma_start(out_d[qt * 128:(qt + 1) * 128,
                                            1024:2048], ob[:, 1024:2048])
                    obs.pop(qt)

            with (
                tc.tile_pool(name="ex", bufs=3) as exp_pool,
                tc.tile_pool(name="ac", bufs=3) as acp,
                tc.tile_pool(name="rp", bufs=2) as rp,
                tc.tile_pool(name="ob", bufs=2) as obp,
                tc.tile_pool(name="ps2", bufs=2, space="PSUM") as ps2,
                tc.tile_pool(name="psm", bufs=1, space="PSUM") as psm,
                tc.tile_pool(name="psw", bufs=1, space="PSUM") as psw,
                tc.tile_pool(name="ps3", bufs=2, space="PSUM") as ps3,
            ):
                for qc in range(QC):
                    for h in range(HPC):
                        qT = qkT[h][qc]
                        kT_h = qkT[2 + h]
                        wv_ps = psw.tile([128, 512], F32, tag="wv", bufs=1,
                                         name=f"wvps{h}_{qc}")
                        exs = {}
                        acc = None
                        lg_cur = None
                        for st in range(ST + 2):
                            if st < ST:
                                pair, side = st // 2, st % 2
                                if side == 0:
                                    lg_cur = ps2.tile(
                                        [128, 1024], F32, tag="lg", bufs=2,
                                        name=f"lg{h}_{qc}_{pair}")
                                kts = kT_h[st // 4][:, (st % 4) * 128:
                                                    (st % 4 + 1) * 128]
                                nc.tensor.matmul(
                                    lg_cur[:, side * 512:(side + 1) * 512],
                                    kts, qT[:], start=True, stop=True)
                                if side == 1:
                                    ex = exp_pool.tile([128, 1024], BF16,
                                                       tag="ex", bufs=3,
                                                       name=f"ex{h}_{qc}_{pair}")
                                    nc.scalar.activation(ex[:], lg_cur[:],
                                                         AF.Exp)
                                    exs[pair] = ex
                                    # non-inplace running sum of the exp
                                    # pair tiles (chain of 7 adds, no copy)
                                    if pair == 1:
                                        nacc = acp.tile(
                                            [128, 1024], BF16, tag="acc",
                                            bufs=3, name=f"acc{h}_{qc}_1")
                                        nc.vector.tensor_add(
                                            nacc[:], exs[0][:], ex[:])
                                        acc = nacc
                                    elif pair > 1:
                                        nacc = acp.tile(
                                            [128, 1024], BF16, tag="acc",
                                            bufs=3,
                                            name=f"acc{h}_{qc}_{pair}")
                                        nc.vector.tensor_add(
                                            nacc[:], acc[:], ex[:])
                                        acc = nacc
                            if st >= 2:
                                sp = st - 2
                                ex = exs[sp // 2]
                                exh = ex[:, (sp % 2) * 512:
                                         (sp % 2 + 1) * 512]
                                nc.tensor.matmul(
                                    wv_ps[:],
                                    vnat[sp][:, h * 128:(h + 1) * 128],
                                    exh, start=(sp == 0), stop=(sp == ST - 1))
                                if sp % 2 == 1:
                                    exs.pop(sp // 2, None)
                                if st % 2 == 1:
                                    emit_slot(ps3, 2, obp)
                            if st == ST + 1:
                                sm = psm.tile([128, 512], F32, tag="sm",
                                              bufs=1, name=f"sm{h}_{qc}")
                                nc.tensor.matmul(sm[:], ones_sb[:],
                                                 acc[:, 0:512],
                                                 start=True, stop=False)
                                nc.tensor.matmul(sm[:], ones_sb[:],
                                                 acc[:, 512:1024],
                                                 start=False, stop=True)
                                rc = rp.tile([128, 512], F32, tag="rc",
                                             bufs=2, name=f"rc{h}_{qc}")
                                nc.vector.reciprocal_approx_fast(rc[:], sm[:])
                        nc.vector.tensor_mul(wvn[h][qc][:], wv_ps[:], rc[:])
                    ph3.extend((qc * 4 + j, oc)
                               for j in range(4) for oc in range(4))

            # final drain: attention PSUM pools are closed, so the last
            # query block's out-projections get a deep bank rotation
            assert not obs, "ob tile straddles the drain boundary"
            with (
                tc.tile_pool(name="obd", bufs=2) as obd,
                tc.tile_pool(name="psd", bufs=6, space="PSUM") as psd,
            ):
                while ph3:
                    emit_slot(psd, 6, obd)

    nc.compile()
    return nc


def kernel(x, qkv_weight, qkv_bias, attn_out_weight, attn_out_bias,
           position_ids):
    global _BUILT, _LAST_RESULTS
    x = np.asarray(x, np.float32)
    qkv_weight = np.asarray(qkv_weight, np.float32)
    qkv_bias = np.asarray(qkv_bias, np.float32)
    attn_out_weight = np.asarray(attn_out_weight, np.float32)
    attn_out_bias = np.asarray(attn_out_bias, np.float32)
    position_ids = np.asarray(position_ids)

    half = HD // 2
    xT = np.ascontiguousarray(x[:, 0, :].T.astype(np.float16))
    inv_freq = 1.0 / (10000.0 ** (np.arange(0, half, 2, dtype=np.float32) / half))
    pos1 = position_ids[0, 0, :].astype(np.float32)
    pos2 = position_ids[0, 1, :].astype(np.float32)
    ang1 = np.concatenate([inv_freq[:, None] * pos1[None, :]] * 2, axis=0)
    ang2 = np.concatenate([inv_freq[:, None] * pos2[None, :]] * 2, axis=0)
    COS = np.concatenate([np.cos(ang1), np.cos(ang2)], axis=0)
    SIN = np.concatenate([np.sin(ang1), np.sin(ang2)], axis=0)
    # fold the rotate-half sign into sin: rows 32:64 and 96:128 negated
    sign = np.ones((128, 1), np.float32)
    sign[32:64] = -1.0
    sign[96:128] = -1.0
    SINP = np.ascontiguousarray((SIN * sign).astype(np.float16))
    COS = np.ascontiguousarray(COS.astype(np.float16))

    in_maps = []
    for c in range(NCORES):
        c0 = c * HPC * HD                     # first q column of this core
        wq = qkv_weight[:, c0:c0 + HPC * HD]
        wk = qkv_weight[:, D + c0:D + c0 + HPC * HD]
        wv = qkv_weight[:, 2 * D + c0:2 * D + c0 + HPC * HD]
        bq = qkv_bias[c0:c0 + HPC * HD]
        bk = qkv_bias[D + c0:D + c0 + HPC * HD]
        wo = attn_out_weight[c0:c0 + HPC * HD, :]
        wqk = np.concatenate([wq, wk], axis=1).astype(np.float16)  # [D, 512]
        # pack stationary tiles per-m: [128, 4*2048]
        wqkP = np.concatenate(
            [np.concatenate([wqk[k * 128:(k + 1) * 128,
                                 m * 128:(m + 1) * 128]
                             for k in range(KT)], axis=1)
             for m in range(4)], axis=1)
        wvf = wv.astype(np.float16)
        wvP = np.concatenate(
            [np.concatenate([wvf[k * 128:(k + 1) * 128,
                                 h * 128:(h + 1) * 128]
                             for k in range(KT)], axis=1)
             for h in range(HPC)], axis=1)
        bqk = np.ascontiguousarray(
            np.stack([bq[:128], bq[128:], bk[:128], bk[128:]], axis=1))
        in_maps.append({
            "xT": xT,
            "wqk": np.ascontiguousarray(wqkP),
            "bqk": bqk,
            "wv": np.ascontiguousarray(wvP),
            "cos": COS,
            "sin": SINP,
            "wo": np.ascontiguousarray(wo.astype(ml_dtypes.bfloat16)),
        })

    if _BUILT is None:
        _BUILT = _build()
    res = run_bass_kernel_spmd(_BUILT, in_maps, core_ids=list(range(NCORES)))
    _LAST_RESULTS = res

    acc = np.zeros((S, D), dtype=np.float32)
    for r in res.results:
        acc += r["out"].astype(np.float32)
    bv = qkv_bias[2 * D:3 * D]
    acc += (bv @ attn_out_weight)[None, :] + attn_out_bias[None, :]
    return acc.reshape(S, 1, D).astype(np.float32)


# revision 25
# speedup vs baseline: 1.0678x; 1.0063x over previous
"""GLM-style dual-RoPE attention block on 8 trn2 NeuronCores.

Sharding: tensor-parallel over heads (16 heads -> 2 per core).
Per core: QKV projection for its heads (transposed layout), dual RoPE,
full S x S attention (streamed softmax over key tiles, no max subtraction
-- max |logit| ~60 so exp stays in bf16 range), unnormalized P@V,
late normalization, and a partial output projection.  Partials are summed
on host; qkv v-bias is folded into a host-side constant row, attn_out
bias added on host.

v2 restructure vs the 228us baseline:
- Phase 1 runs m-outer / k-inner: each of the 6 qkv output groups
  accumulates over all 16 contraction tiles before the next group
  starts, so PSUM evictions trail one group behind the matmul stream
  and the 2-3us round-boundary stalls disappear.  Weights are packed
  per-m on host so a group's stationary tiles arrive in 2 big DMAs.
- x tiles stream on two DMA queues (sync + gpsimd) to double round-0
  fill bandwidth; cos/sin/wo arrive later, off the critical window.
- Attention uses 512-query blocks (4 qc x 2 heads).  Logits go to
  [128,1024] f32 PSUM pairs (two key tiles), one Exp per pair, the
  bf16 exp accumulation is split across vector (even tile) and gpsimd
  (odd tile) with the two partial sums folded by an accumulating
  ones-matmul pair.  PV lags logits by 2 tiles.
- Output projection slots (qt, oc) are popped one per st-pair as soon
  as a query block's two heads are normalized, so only the last query
  block's 16 slots drain at the end (vs 36 before); drain evictions
  alternate vector/scalar.
"""

import ml_dtypes
import numpy as np

import concourse.bass as bass
from concourse import bacc
import concourse.mybir as mybir
import concourse.tile as tile
from concourse.bass_utils import run_bass_kernel_spmd
from concourse.masks import make_identity

F32 = mybir.dt.float32
BF16 = mybir.dt.bfloat16
F16 = mybir.dt.float16
AF = mybir.ActivationFunctionType
OP = mybir.AluOpType

S, D, H, HD = 2048, 2048, 16, 128
NCORES = 8
HPC = H // NCORES          # heads per core = 2
KT = D // 128              # 16 contraction tiles
ST = S // 128              # 16 sequence 128-tiles
QC = S // 512              # 4 sequence 512-chunks

_LAST_RESULTS = None
_BUILT = None


def _build():
    nc = bacc.Bacc("TRN2", target_bir_lowering=False, debug=False,
                   num_devices=NCORES)
    xT_d = nc.dram_tensor("xT", [D, S], F16, kind="ExternalInput").ap()
    # wqk packed per-m: [128, 4*2048], m-th slice's col k*128.. holds
    # W[k*128:(k+1)*128, m-slice] (stationary tiles contiguous per m)
    wqk_d = nc.dram_tensor("wqk", [128, 4 * D], F16, kind="ExternalInput").ap()
    bqk_d = nc.dram_tensor("bqk", [128, 4], F32, kind="ExternalInput").ap()
    wv_d = nc.dram_tensor("wv", [128, HPC * D], F16, kind="ExternalInput").ap()
    cos_d = nc.dram_tensor("cos", [128, S], F16, kind="ExternalInput").ap()
    sin_d = nc.dram_tensor("sin", [128, S], F16, kind="ExternalInput").ap()
    wo_d = nc.dram_tensor("wo", [HPC * 128, D], BF16, kind="ExternalInput").ap()
    out_d = nc.dram_tensor("out", [S, D], BF16, kind="ExternalOutput").ap()

    with tile.TileContext(nc) as tc:
        with tc.tile_pool(name="res", bufs=1) as res:
            cos_sb = res.tile([128, S], F16, tag="cos")
            sin_sb = res.tile([128, S], F16, tag="sin")
            bqk_sb = res.tile([128, 4], F32, tag="bqk")
            wo_sb = [res.tile([128, D], BF16, tag=f"wo{h}", name=f"wo{h}")
                     for h in range(HPC)]
            qkT = [[res.tile([128, 512], F16, tag=f"qkT{m}_{nq}",
                             name=f"qkT{m}_{nq}") for nq in range(QC)]
                   for m in range(4)]
            vnat = [res.tile([128, 256], BF16, tag=f"vnat{st}",
                             name=f"vnat{st}") for st in range(ST)]
            wvn = [[res.tile([128, 512], BF16, tag=f"wvn{h}_{nq}",
                             name=f"wvn{h}_{nq}") for nq in range(QC)]
                   for h in range(HPC)]

            # ---- phase 1: qkv^T = W^T @ x^T, m-outer ----
            with (
                tc.tile_pool(name="xs", bufs=1) as xs,
                tc.tile_pool(name="tmp", bufs=3) as tmp,
                tc.tile_pool(name="vt", bufs=1) as vtp,
                tc.tile_pool(name="ps1", bufs=5, space="PSUM") as ps1,
                tc.tile_pool(name="pst", bufs=2, space="PSUM") as pst,
            ):
                # m=0 in quarters so the very first matmul only waits on
                # a 128KB transfer; m=1..3 in halves
                wqk_sb = [[xs.tile([128, 512 if m == 0 else 1024], F16,
                                   tag=f"wqk{m}_{ha}",
                                   name=f"wqk{m}_{ha}", bufs=1)
                           for ha in range(4 if m == 0 else 2)]
                          for m in range(4)]
                wv_sb = [[xs.tile([128, 1024], F16, tag=f"wvw{h}_{ha}",
                                  name=f"wvw{h}_{ha}", bufs=1)
                          for ha in range(2)] for h in range(HPC)]
                vT = [[vtp.tile([128, 512], BF16, tag=f"vT{h}_{nq}",
                                name=f"vT{h}_{nq}") for nq in range(QC)]
                      for h in range(HPC)]
                ones_f = res.tile([128, 128], F32, tag="ones_f")
                nc.gpsimd.memset(ones_f[:], 1.0)
                ones_sb = res.tile([128, 128], BF16, tag="ones")
                nc.vector.tensor_copy(ones_sb[:], ones_f[:])
                ident_f = res.tile([128, 128], F32, tag="ident_f")
                make_identity(nc, ident_f[:])
                ident = res.tile([128, 128], BF16, tag="ident")
                nc.vector.tensor_copy(ident[:], ident_f[:])

                def rope_evict(m, nq, ps):
                    zb = tmp.tile([128, 512], F16, tag="zb", bufs=6,
                                  name=f"zb{m}_{nq}")
                    nc.vector.tensor_scalar_add(zb[:], ps[:],
                                                bqk_sb[:, m:m + 1])
                    return zb

                def rope_finish(m, nq, zb):
                    ns = slice(nq * 512, (nq + 1) * 512)
                    ts = tmp.tile([128, 512], F16, tag="ts", bufs=6,
                                  name=f"ts{m}_{nq}")
                    nc.vector.tensor_mul(ts[:], zb[:], sin_sb[:, ns])
                    tp = tmp.tile([128, 512], F16, tag="tp", bufs=6,
                                  name=f"tp{m}_{nq}")
                    for blk in range(2):
                        b0 = blk * 64
                        nc.gpsimd.dma_start(tp[b0:b0 + 32, :],
                                            ts[b0 + 32:b0 + 64, :])
                        nc.gpsimd.dma_start(tp[b0 + 32:b0 + 64, :],
                                            ts[b0:b0 + 32, :])
                    t1 = tmp.tile([128, 512], F16, tag="t1",
                                  name=f"t1{m}_{nq}")
                    nc.vector.tensor_mul(t1[:], zb[:], cos_sb[:, ns])
                    nc.vector.tensor_add(qkT[m][nq][:], t1[:], tp[:])

                xts = {}

                def issue_xt(nq, k):
                    t = xs.tile([128, 512], F16, tag="xt", bufs=32,
                                name=f"xt{nq}_{k}")
                    q = nc.sync if (k % 2 == 0) else nc.gpsimd
                    q.dma_start(t[:], xT_d[k * 128:(k + 1) * 128,
                                           nq * 512:(nq + 1) * 512])
                    xts[(nq, k)] = t

                def _emit_T(h, nq):
                    for j in range(4):
                        st = nq * 4 + j
                        tp2 = pst.tile([128, 128], BF16, tag="tp2", bufs=2,
                                       name=f"tp2_{h}_{st}")
                        nc.tensor.transpose(
                            tp2[:], vT[h][nq][:, j * 128:(j + 1) * 128],
                            ident[:])
                        nc.scalar.copy(vnat[st][:, h * 128:(h + 1) * 128],
                                       tp2[:])

                pending_T = []
                deferred_rope = []

                for nq in range(QC):
                    for k in range(KT):
                        issue_xt(nq, k)
                    if nq == 0:
                        # bqk first (needed at the first evict); weights
                        # on the scalar queue, rope tables on gpsimd
                        # (after its xt issues) -- only sync/scalar/
                        # gpsimd can host DMA rings
                        for m in range(4):
                            npiece = 4 if m == 0 else 2
                            w = 2048 // npiece
                            for ha in range(npiece):
                                nc.scalar.dma_start(
                                    wqk_sb[m][ha][:],
                                    wqk_d[:, m * D + ha * w:
                                          m * D + (ha + 1) * w])
                                if m == 0 and ha == 0:
                                    # tiny; needed first at m0's evict,
                                    # must not gate the first matmul
                                    nc.scalar.dma_start(bqk_sb[:],
                                                        bqk_d[:, :])
                        for h in range(HPC):
                            for ha in range(2):
                                nc.scalar.dma_start(
                                    wv_sb[h][ha][:],
                                    wv_d[:, h * D + ha * 1024:
                                         h * D + (ha + 1) * 1024])
                        nc.gpsimd.dma_start(cos_sb[:], cos_d[:, :])
                        nc.gpsimd.dma_start(sin_sb[:], sin_d[:, :])
                    if nq == 1:
                        for h in range(HPC):
                            nc.gpsimd.dma_start(
                                wo_sb[h][:], wo_d[h * 128:(h + 1) * 128, :])
                    # last round runs v-groups first so the vT copies,
                    # transposes and vnat evictions finish ~10us before
                    # the attention seam instead of right at it
                    # (cos/sin/bqk/weight DMAs were all emitted in round
                    # 0, so program order stays write-before-read)
                    morder = (4, 5, 2, 3, 0, 1) if nq == 3 else range(6)
                    for m in morder:
                        ps = ps1.tile([128, 512], F32, tag="qkv", bufs=5,
                                      name=f"qkv{nq}_{m}")
                        for k in range(KT):
                            if m == 0:
                                w = wqk_sb[0][k // 4][:, (k % 4) * 128:
                                                      (k % 4 + 1) * 128]
                            elif m < 4:
                                w = wqk_sb[m][k // 8][:, (k % 8) * 128:
                                                      (k % 8 + 1) * 128]
                            else:
                                w = wv_sb[m - 4][k // 8][:, (k % 8) * 128:
                                                         (k % 8 + 1) * 128]
                            nc.tensor.matmul(ps[:], w, xts[(nq, k)][:],
                                             start=(k == 0), stop=(k == KT - 1))
                        # flush v-transposes pending from the previous
                        # group so their scalar copies had time to land
                        for t_args in pending_T:
                            _emit_T(*t_args)
                        pending_T = []
                        if m < 4:
                            zb = rope_evict(m, nq, ps)
                            if nq == 0:
                                deferred_rope.append((m, nq, zb))
                            else:
                                rope_finish(m, nq, zb)
                        else:
                            h = m - 4
                            nc.scalar.copy(vT[h][nq][:], ps[:])
                            pending_T.append((h, nq))
                    if nq == 0:
                        # round 0: rope tables may still be in flight
                        # during the early groups; run theropes after
                        # the round's matmuls so vector work overlaps
                        # round 1's stream
                        for args in deferred_rope:
                            rope_finish(*args)
                        deferred_rope = []

                for t_args in pending_T:
                    _emit_T(*t_args)
                pending_T = []

            # ---- phase 2 + 3: attention and out-projection ----
            ph3 = []
            obs = {}
            nslot = [0]

            def emit_slot(oppool, opbufs, obpool):
                if not ph3:
                    return
                qt, oc = ph3.pop(0)
                if oc == 0:
                    obs[qt] = obpool.tile([128, D], BF16, tag="ob",
                                          name=f"ob{qt}")
                ob = obs[qt]
                op = oppool.tile([128, 512], F32, tag="op", bufs=opbufs,
                                 name=f"op{qt}_{oc}")
                for h2 in range(HPC):
                    nc.tensor.matmul(
                        op[:],
                        wvn[h2][qt // 4][:, (qt % 4) * 128:
                                         (qt % 4 + 1) * 128],
                        wo_sb[h2][:, oc * 512:(oc + 1) * 512],
                        start=(h2 == 0), stop=(h2 == HPC - 1))
                n = nslot[0]
                nslot[0] += 1
                use_scalar = (n % 2 == 1)
                dst = ob[:, oc * 512:(oc + 1) * 512]
                if use_scalar:
                    nc.scalar.copy(dst, op[:])
                else:
                    nc.vector.tensor_copy(dst, op[:])
                if oc == 1:
                    nc.sync.dma_start(out_d[qt * 128:(qt + 1) * 128,
                                            0:1024], ob[:, 0:1024])
                elif oc == 3:
                    nc.sync.dma_start(out_d[qt * 128:(qt + 1) * 128,
                                            1024:2048], ob[:, 1024:2048])
                    obs.pop(qt)

            with (
                tc.tile_pool(name="ex", bufs=4) as exp_pool,
                tc.tile_pool(name="ac", bufs=4) as acp,
                tc.tile_pool(name="rp", bufs=2) as rp,
                tc.tile_pool(name="ob", bufs=2) as obp,
                tc.tile_pool(name="ps2", bufs=2, space="PSUM") as ps2,
                tc.tile_pool(name="psm", bufs=1, space="PSUM") as psm,
                tc.tile_pool(name="psw", bufs=1, space="PSUM") as psw,
                tc.tile_pool(name="ps3", bufs=2, space="PSUM") as ps3,
            ):
                for qc in range(QC):
                    for h in range(HPC):
                        qT = qkT[h][qc]
                        kT_h = qkT[2 + h]
                        wv_ps = psw.tile([128, 512], F32, tag="wv", bufs=1,
                                         name=f"wvps{h}_{qc}")
                        exs = {}
                        acc = None
                        lg_cur = None
                        for st in range(ST + 2):
                            if st < ST:
                                pair, side = st // 2, st % 2
                                if side == 0:
                                    lg_cur = ps2.tile(
                                        [128, 1024], F32, tag="lg", bufs=2,
                                        name=f"lg{h}_{qc}_{pair}")
                                kts = kT_h[st // 4][:, (st % 4) * 128:
                                                    (st % 4 + 1) * 128]
                                nc.tensor.matmul(
                                    lg_cur[:, side * 512:(side + 1) * 512],
                                    kts, qT[:], start=True, stop=True)
                                if side == 1:
                                    ex = exp_pool.tile([128, 1024], BF16,
                                                       tag="ex", bufs=4,
                                                       name=f"ex{h}_{qc}_{pair}")
                                    nc.scalar.activation(ex[:], lg_cur[:],
                                                         AF.Exp)
                                    exs[pair] = ex
                                    # non-inplace running sum of the exp
                                    # pair tiles (chain of 7 adds, no copy)
                                    if pair == 1:
                                        nacc = acp.tile(
                                            [128, 1024], BF16, tag="acc",
                                            bufs=4, name=f"acc{h}_{qc}_1")
                                        nc.vector.tensor_add(
                                            nacc[:], exs[0][:], ex[:])
                                        acc = nacc
                                    elif pair > 1:
                                        nacc = acp.tile(
                                            [128, 1024], BF16, tag="acc",
                                            bufs=4,
                                            name=f"acc{h}_{qc}_{pair}")
                                        nc.vector.tensor_add(
                                            nacc[:], acc[:], ex[:])
                                        acc = nacc
                            if st >= 2:
                                sp = st - 2
                                ex = exs[sp // 2]
                                exh = ex[:, (sp % 2) * 512:
                                         (sp % 2 + 1) * 512]
                                nc.tensor.matmul(
                                    wv_ps[:],
                                    vnat[sp][:, h * 128:(h + 1) * 128],
                                    exh, start=(sp == 0), stop=(sp == ST - 1))
                                if sp % 2 == 1:
                                    exs.pop(sp // 2, None)
                                if st % 2 == 1:
                                    emit_slot(ps3, 2, obp)
                            if st == ST + 1:
                                sm = psm.tile([128, 512], F32, tag="sm",
                                              bufs=1, name=f"sm{h}_{qc}")
                                nc.tensor.matmul(sm[:], ones_sb[:],
                                                 acc[:, 0:512],
                                                 start=True, stop=False)
                                nc.tensor.matmul(sm[:], ones_sb[:],
                                                 acc[:, 512:1024],
                                                 start=False, stop=True)
                                rc = rp.tile([128, 512], F32, tag="rc",
                                             bufs=2, name=f"rc{h}_{qc}")
                                nc.vector.reciprocal_approx_fast(rc[:], sm[:])
                        nc.vector.tensor_mul(wvn[h][qc][:], wv_ps[:], rc[:])
                    ph3.extend((qc * 4 + j, oc)
                               for j in range(4) for oc in range(4))

            # final drain: attention PSUM pools are closed, so the last
            # query block's out-projections get a deep bank rotation
            assert not obs, "ob tile straddles the drain boundary"
            with (
                tc.tile_pool(name="obd", bufs=2) as obd,
                tc.tile_pool(name="psd", bufs=6, space="PSUM") as psd,
            ):
                while ph3:
                    emit_slot(psd, 6, obd)

    nc.compile()
    return nc


def kernel(x, qkv_weight, qkv_bias, attn_out_weight, attn_out_bias,
           position_ids):
    global _BUILT, _LAST_RESULTS
    x = np.asarray(x, np.float32)
    qkv_weight = np.asarray(qkv_weight, np.float32)
    qkv_bias = np.asarray(qkv_bias, np.float32)
    attn_out_weight = np.asarray(attn_out_weight, np.float32)
    attn_out_bias = np.asarray(attn_out_bias, np.float32)
    position_ids = np.asarray(position_ids)

    half = HD // 2
    xT = np.ascontiguousarray(x[:, 0, :].T.astype(np.float16))
    inv_freq = 1.0 / (10000.0 ** (np.arange(0, half, 2, dtype=np.float32) / half))
    pos1 = position_ids[0, 0, :].astype(np.float32)
    pos2 = position_ids[0, 1, :].astype(np.float32)
    ang1 = np.concatenate([inv_freq[:, None] * pos1[None, :]] * 2, axis=0)
    ang2 = np.concatenate([inv_freq[:, None] * pos2[None, :]] * 2, axis=0)
    COS = np.concatenate([np.cos(ang1), np.cos(ang2)], axis=0)
    SIN = np.concatenate([np.sin(ang1), np.sin(ang2)], axis=0)
    # fold the rotate-half sign into sin: rows 32:64 and 96:128 negated
    sign = np.ones((128, 1), np.float32)
    sign[32:64] = -1.0
    sign[96:128] = -1.0
    SINP = np.ascontiguousarray((SIN * sign).astype(np.float16))
    COS = np.ascontiguousarray(COS.astype(np.float16))

    in_maps = []
    for c in range(NCORES):
        c0 = c * HPC * HD                     # first q column of this core
        wq = qkv_weight[:, c0:c0 + HPC * HD]
        wk = qkv_weight[:, D + c0:D + c0 + HPC * HD]
        wv = qkv_weight[:, 2 * D + c0:2 * D + c0 + HPC * HD]
        bq = qkv_bias[c0:c0 + HPC * HD]
        bk = qkv_bias[D + c0:D + c0 + HPC * HD]
        wo = attn_out_weight[c0:c0 + HPC * HD, :]
        wqk = np.concatenate([wq, wk], axis=1).astype(np.float16)  # [D, 512]
        # pack stationary tiles per-m: [128, 4*2048]
        wqkP = np.concatenate(
            [np.concatenate([wqk[k * 128:(k + 1) * 128,
                                 m * 128:(m + 1) * 128]
                             for k in range(KT)], axis=1)
             for m in range(4)], axis=1)
        wvf = wv.astype(np.float16)
        wvP = np.concatenate(
            [np.concatenate([wvf[k * 128:(k + 1) * 128,
                                 h * 128:(h + 1) * 128]
                             for k in range(KT)], axis=1)
             for h in range(HPC)], axis=1)
        bqk = np.ascontiguousarray(
            np.stack([bq[:128], bq[128:], bk[:128], bk[128:]], axis=1))
        in_maps.append({
            "xT": xT,
            "wqk": np.ascontiguousarray(wqkP),
            "bqk": bqk,
            "wv": np.ascontiguousarray(wvP),
            "cos": COS,
            "sin": SINP,
            "wo": np.ascontiguousarray(wo.astype(ml_dtypes.bfloat16)),
        })

    if _BUILT is None:
        _BUILT = _build()
    res = run_bass_kernel_spmd(_BUILT, in_maps, core_ids=list(range(NCORES)))
    _LAST_RESULTS = res

    acc = np.zeros((S, D), dtype=np.float32)
    for r in res.results:
        acc += r["out"].astype(np.float32)
    bv = qkv_bias[2 * D:3 * D]
    acc += (bv @ attn_out_weight)[None, :] + attn_out_bias[None, :]
    return acc.reshape(S, 1, D).astype(np.float32)
